# revision 22
# baseline (speedup 1.0000x reference)
"""FFT_Net Trainium2 kernel.

Per (batch, channel): Range DFT (512) then Doppler DFT (256) as complex
GEMMs on the TensorEngine, followed by InstanceNorm fused on the
vector/scalar engines. Data-parallel over the batch dim across 8 cores.

Key tricks:
- Stage 1 exploits the FFT structure: 512-DFT = radix-2 butterfly
  (vector engine, fp16) + two twiddle-folded 256x256 complex GEMMs
  (one per output parity), halving TensorEngine work vs the dense DFT.
  The row permutation (k = k1 + 2*k2) is undone for free by a strided
  output-DMA view.
- Complex GEMMs use the 4-multiplication form with negated imaginary
  weights so both real/imag accumulate entirely in PSUM: no vector-
  engine combine arithmetic, just PSUM->SBUF copies.
- Stage 2 streams concatenated weights [Wr|Wi] so one accumulation
  group yields [zr | zi] in a single PSUM bank.
- InstanceNorm mean needs no reduction: the mean over an instance is
  exactly the DC element of the input (DFT rows sum to N*delta_0).
- Sum-of-squares is one big Square-activation accumulation pass per
  component on the scalar engine; one GpSimd partition_all_reduce
  yields partition-replicated stats so normalize needs no broadcast.
- Matmul operands are fp16 (PSUM accumulation stays fp32).
- Per-(b,c) stats/normalize chains are emitted one iteration behind
  the GEMM stream so the TensorEngine never waits on them.

kernel(**inputs) takes the FULL inputs and returns the FULL output.
"""
import sys

sys.path.insert(0, "/opt/trn_rl_repo")

import numpy as np
import ml_dtypes

import concourse.bass as bass  # noqa: F401
import concourse.tile as tile
from concourse import bacc, bass_isa, mybir
from concourse.bass_utils import run_bass_kernel_spmd

B, C, R, D = 16, 16, 512, 256
NCORES = 8
BS = B // NCORES  # batches per core
EPS = 1e-5
N_NORM = R * D
F32 = mybir.dt.float32
F16 = mybir.dt.bfloat16  # bf16: DVE 2-input ops avoid the port-read halving
MULT = mybir.AluOpType.mult
ADD = mybir.AluOpType.add
SUB = mybir.AluOpType.subtract
SQRT = mybir.ActivationFunctionType.Sqrt
SQUARE = mybir.ActivationFunctionType.Square
IDENT = mybir.ActivationFunctionType.Identity


def build():
    nc = bacc.Bacc(None, target_bir_lowering=False)

    xr_d = nc.dram_tensor("x_real", [BS, C, R, D], F16, kind="ExternalInput")
    xi_d = nc.dram_tensor("x_imag", [BS, C, R, D], F16, kind="ExternalInput")
    # twiddle-folded stage-1 matrices, one complex [256,256] per parity k1
    m_d = {}
    for k1 in range(2):
        for part in ("r", "i", "ni"):
            nm = f"M{k1}{part}"
            m_d[(k1, part)] = nc.dram_tensor(nm, [256, 256], F16,
                                             kind="ExternalInput")
    # stage-2 twiddle-folded matrices per parity k1p (all [128, 256]):
    # WA = [M2r | M2i], WB = [-M2i | M2r]
    w2_d = {}
    for k1p in range(2):
        for nm in ("A", "B"):
            w2_d[(k1p, nm)] = nc.dram_tensor(f"W2{nm}{k1p}", [128, 256], F16,
                                             kind="ExternalInput")
    out_d = nc.dram_tensor("out", [BS, 2 * C, R, D], F32, kind="ExternalOutput")

    with tile.TileContext(nc) as tc:
        with tc.tile_pool(name="wpool", bufs=1) as wpool, \
             tc.tile_pool(name="xpool", bufs=4) as xpool, \
             tc.tile_pool(name="apool", bufs=3) as apool, \
             tc.tile_pool(name="ypool", bufs=2) as ypool, \
             tc.tile_pool(name="zpool", bufs=4) as zpool, \
             tc.tile_pool(name="stpool", bufs=6) as stpool, \
             tc.tile_pool(name="sqpool", bufs=2) as sqpool, \
             tc.tile_pool(name="pspool", bufs=1, space="PSUM") as pspool:

            # --- weights, resident for the whole kernel ---
            m1w = {}
            w256 = {}
            for nm, dram, shape, eng, store, key in (
                    ("m0r", m_d[(0, "r")], [128, 2, 256], nc.scalar, m1w, (0, "r")),
                    ("m0i", m_d[(0, "i")], [128, 2, 256], nc.scalar, m1w, (0, "i")),
                    ("m0ni", m_d[(0, "ni")], [128, 2, 256], nc.scalar, m1w, (0, "ni")),
                    ("m1r", m_d[(1, "r")], [128, 2, 256], nc.gpsimd, m1w, (1, "r")),
                    ("m1i", m_d[(1, "i")], [128, 2, 256], nc.gpsimd, m1w, (1, "i")),
                    ("m1ni", m_d[(1, "ni")], [128, 2, 256], nc.gpsimd, m1w, (1, "ni"))):
                t = wpool.tile(shape, F16, name=f"w_{nm}")
                eng.dma_start(
                    out=t,
                    in_=dram[:].rearrange("(k p) n -> p k n", p=128))
                store[key] = t
            for (k1p, nm), dram in w2_d.items():
                t = wpool.tile([128, 256], F16, name=f"w2{nm}{k1p}")
                nc.gpsimd.dma_start(out=t, in_=dram[:])
                w256[(k1p, nm)] = t
            eps128 = wpool.tile([128, 1], F32, name="eps128")
            nc.vector.memset(eps128, EPS)
            # all-ones fp32 [128,128]: one matmul does the cross-partition
            # stats reduction AND replicates the result to every partition,
            # keeping GpSimd to a single ucode library (no LIBRARY_RELOAD
            # stalls from partition_all_reduce).
            ones_f32 = wpool.tile([128, 128], F32, name="ones_f32")
            nc.vector.memset(ones_f32, 1.0)

            def emit_loads(b, c):
                """Input DMAs for one (b, c) — first on the sync queue so
                stores never head-of-line block prefetch."""
                xr = xpool.tile([128, 4, 256], F16, name="xr", tag="xr")
                nc.sync.dma_start(
                    out=xr,
                    in_=xr_d[b, c].rearrange("(k p) d -> p k d", p=128))
                xi = xpool.tile([128, 4, 256], F16, name="xi", tag="xi")
                nc.sync.dma_start(
                    out=xi,
                    in_=xi_d[b, c].rearrange("(k p) d -> p k d", p=128))
                return xr, xi

            def emit_front(b, c, xr, xi):
                """Stage-1 for one (b, c): radix-2 butterfly (GpSimd only —
                keeps its ucode queue monotonic so it runs iterations ahead),
                stage-1 folded GEMMs, then fused PSUM-drain + stage-2
                butterfly on the vector engine (Bt = ps[dc0] -+ ps[dc1])."""
                # partials cols: 0 q_r, 1 q_i, 2-3 DC mean
                partials = stpool.tile([128, 4], F32, name="partials",
                                       tag="partials")
                nc.vector.memset(partials[:, 2:4], 0.0)
                nc.vector.tensor_copy(out=partials[0:1, 2:3],
                                      in_=xr[0:1, 0, 0:1])
                nc.vector.tensor_copy(out=partials[0:1, 3:4],
                                      in_=xi[0:1, 0, 0:1])

                # --- radix-2 butterfly along R: A0 = xlo+xhi, A1 = xlo-xhi
                A = {}
                for comp, src in (("r", xr), ("i", xi)):
                    a0 = apool.tile([128, 2, 256], F16, name=f"a0{comp}",
                                    tag=f"a0{comp}")
                    nc.gpsimd.tensor_add(out=a0, in0=src[:, 0:2, :],
                                         in1=src[:, 2:4, :])
                    A[(0, comp)] = a0
                    a1 = apool.tile([128, 2, 256], F16, name=f"a1{comp}",
                                    tag=f"a1{comp}")
                    nc.gpsimd.tensor_sub(out=a1, in0=src[:, 0:2, :],
                                         in1=src[:, 2:4, :])
                    A[(1, comp)] = a1

                # --- stage 1 GEMMs + PSUM drains ---
                # ycomb[dc] free layout: [k1, r-half | i-half]
                ycomb = {}
                for dc in range(2):
                    ycomb[dc] = ypool.tile(
                        [128, 2, 512], F16, name=f"yc{dc}", tag=f"yc{dc}")
                for k1 in range(2):
                    ps1k = pspool.tile([128, 2, 512], F32, name="ps1",
                                       tag="ps1", bufs=2)
                    for dc in range(2):
                        # real half: Ar@Mr + Ai@(-Mi)
                        for src_c, wpart in (("r", "r"), ("i", "ni")):
                            for n2c in range(2):
                                nc.tensor.matmul(
                                    out=ps1k[:, dc, 0:256],
                                    lhsT=A[(k1, src_c)][
                                        :, n2c, dc * 128:(dc + 1) * 128],
                                    rhs=m1w[(k1, wpart)][:, n2c, :],
                                    start=(src_c == "r" and n2c == 0),
                                    stop=(src_c == "i" and n2c == 1))
                        # imag half: Ar@Mi + Ai@Mr
                        for src_c, wpart in (("r", "i"), ("i", "r")):
                            for n2c in range(2):
                                nc.tensor.matmul(
                                    out=ps1k[:, dc, 256:512],
                                    lhsT=A[(k1, src_c)][
                                        :, n2c, dc * 128:(dc + 1) * 128],
                                    rhs=m1w[(k1, wpart)][:, n2c, :],
                                    start=(src_c == "r" and n2c == 0),
                                    stop=(src_c == "i" and n2c == 1))
                    # PSUM drains on vector (fp32 -> fp16 SBUF)
                    for dc in range(2):
                        for h in range(2):
                            nc.vector.tensor_copy(
                                out=ycomb[dc][:, k1,
                                              h * 256:(h + 1) * 256],
                                in_=ps1k[:, dc, h * 256:(h + 1) * 256])

                # --- stage-2 radix-2 butterfly along d: B = y_dc0 -+ y_dc1
                # mostly on GpSimd (it has run-ahead slack and no chain
                # pressure); one on Vector. Bt free layout: [k1, 256].
                Bt = {}
                for k1p, comp in ((0, "r"), (0, "i"), (1, "r"), (1, "i")):
                    h = 0 if comp == "r" else 1
                    bt = apool.tile([128, 2, 256], F16, name=f"b{k1p}{comp}",
                                    tag=f"b{k1p}{comp}")
                    eng = nc.gpsimd if comp == "r" else nc.vector
                    if k1p == 0:
                        eng.tensor_add(out=bt,
                                       in0=ycomb[0][:, :, h * 256:(h + 1) * 256],
                                       in1=ycomb[1][:, :, h * 256:(h + 1) * 256])
                    else:
                        eng.tensor_sub(out=bt,
                                       in0=ycomb[0][:, :, h * 256:(h + 1) * 256],
                                       in1=ycomb[1][:, :, h * 256:(h + 1) * 256])
                    Bt[(k1p, comp)] = bt
                return dict(b=b, c=c, partials=partials, Bt=Bt, xr=xr, xi=xi)

            def emit_back(st):
                """Stage-2 GEMMs + z drains + sumsq for one (b, c)."""
                Bt, partials = st["Bt"], st["partials"]
                z_all = zpool.tile([128, 4, 512], F32, name="z_all",
                                   tag="z_all", bufs=4)
                for m2 in range(4):
                    ps2 = pspool.tile([128, 512], F32, name="ps2",
                                      tag="ps2", bufs=3)
                    # bank layout: [k1p=0: zr|zi (128 each) | k1p=1: zr|zi]
                    # one 4-MM accumulation group per bank: k1p=1's first
                    # write lands on has_written bits cleared by the group
                    # start, so it overwrites correctly.
                    k1c, hc = m2 // 2, m2 % 2
                    for k1p in range(2):
                        nc.tensor.matmul(
                            out=ps2[:, k1p * 256:(k1p + 1) * 256],
                            lhsT=Bt[(k1p, "r")][:, k1c,
                                                hc * 128:(hc + 1) * 128],
                            rhs=w256[(k1p, "A")],
                            start=(k1p == 0), stop=False,
                            skip_group_check=True)
                        nc.tensor.matmul(
                            out=ps2[:, k1p * 256:(k1p + 1) * 256],
                            lhsT=Bt[(k1p, "i")][:, k1c,
                                                hc * 128:(hc + 1) * 128],
                            rhs=w256[(k1p, "B")],
                            start=False, stop=(k1p == 1),
                            skip_group_check=True)
                    # whole-bank PSUM -> SBUF copy on ACT
                    nc.scalar.copy(out=z_all[:, m2, :], in_=ps2)
                # strided per-component views: slot = (k1p, comp, k2)
                zv = z_all.rearrange("p m (k1p comp k2) -> p comp m k1p k2",
                                     k1p=2, comp=2)
                # one big sumsq pass per component on ACT
                for ci in range(2):
                    sq = sqpool.tile([128, 4, 2, 128], F16, name="sq",
                                     tag="sq")
                    nc.scalar.activation(
                        out=sq, in_=zv[:, ci], func=SQUARE,
                        accum_out=partials[:, ci:ci + 1])
                return dict(b=st["b"], c=st["c"], partials=partials,
                            z_all=z_all)

            def emit_stats(st):
                """Deferred per-(b,c): cross-partition sumsq reduce via an
                all-ones matmul (reduces over partitions AND broadcasts the
                result to all 128 partitions in one PE op), stats math as
                cheap 1-input+scalar vector ops, normalize on the scalar
                engine (out = Identity(z*istd - mb)), then store.

                Emitted under high_priority: every op's inputs are a full
                pipeline stage old, so the chain should win scheduler ties
                against same-engine bulk work (drains/butterflies) instead
                of stalling the scalar engine behind them."""
                b, c = st["b"], st["c"]
                partials, z_all = st["partials"], st["z_all"]
                allred = pspool.tile([128, 4], F32, name="psstat",
                                     tag="psstat", bufs=1)
                nc.tensor.matmul(out=allred, lhsT=ones_f32, rhs=partials,
                                 start=True, stop=True)
                # var = E[z^2] - mean^2 ; istd = 1/sqrt(var + eps)
                mean2 = stpool.tile([128, 2], F32, name="mean2", tag="mean2")
                nc.vector.tensor_copy(out=mean2, in_=allred[:, 2:4])
                var2 = stpool.tile([128, 2], F32, name="var2", tag="var2")
                msq = stpool.tile([128, 2], F32, name="msq", tag="msq")
                for ci in range(2):
                    nc.vector.tensor_scalar_mul(
                        out=msq[:, ci:ci + 1], in0=mean2[:, ci:ci + 1],
                        scalar1=mean2[:, ci:ci + 1])
                    nc.vector.tensor_scalar(
                        out=var2[:, ci:ci + 1], in0=allred[:, ci:ci + 1],
                        scalar1=1.0 / N_NORM, scalar2=msq[:, ci:ci + 1],
                        op0=MULT, op1=SUB)
                std2 = stpool.tile([128, 2], F32, name="std2", tag="std2")
                nc.scalar.activation(out=std2, in_=var2, func=SQRT,
                                     bias=eps128, scale=1.0)
                istd = stpool.tile([128, 2], F32, name="istd", tag="istd")
                nc.vector.reciprocal(out=istd, in_=std2)
                nmb = stpool.tile([128, 2], F32, name="nmb", tag="nmb")
                for ci in range(2):
                    nc.vector.tensor_scalar(
                        out=nmb[:, ci:ci + 1], in0=mean2[:, ci:ci + 1],
                        scalar1=istd[:, ci:ci + 1], scalar2=-1.0,
                        op0=MULT, op1=MULT)
                # normalize on ACT (strided de-interleave) and store;
                # stage-1's row permutation (r = k1 + 2*k2, m2 slot =
                # k1*2 + k2c) is undone by the strided DMA view.
                zv = z_all.rearrange("p m (k1p comp k2) -> p comp m k1p k2",
                                     k1p=2, comp=2)
                for ci, comp in enumerate(("r", "i")):
                    # normalize, de-interleaving df = k1p + 2*k2 into natural
                    # column order (d = k2*2 + k1p low bit)
                    z = zpool.tile([128, 4, 256], F32, name=f"z_{comp}",
                                   tag=f"z_{comp}", bufs=3)
                    nc.scalar.activation(
                        out=z.rearrange("p m (k2 k1p) -> p m k1p k2", k1p=2),
                        in_=zv[:, ci], func=IDENT,
                        scale=istd[:, ci:ci + 1], bias=nmb[:, ci:ci + 1])
                    ch = c if comp == "r" else C + c
                    oview = out_d[b, ch].rearrange(
                        "(kc p two) d -> two p kc d", kc=2, two=2)
                    for k1 in range(2):
                        nc.sync.dma_start(
                            out=oview[k1],
                            in_=z[:, 2 * k1:2 * k1 + 2, :])

            # --- software-pipelined emission: loads(i) | stats(i-3) |
            # front(i) | back(i-1) keeps every engine queue free of
            # head-of-line blocking; the stats chain gets a full extra
            # iteration of slack so its cross-engine latency hides.
            SDEPTH = 3
            pairs = [(b, c) for b in range(BS) for c in range(C)]
            fronts = {}
            backs = {}
            for i, (b, c) in enumerate(pairs):
                xr, xi = emit_loads(b, c)
                if i >= SDEPTH:
                    with tc.high_priority():
                        emit_stats(backs.pop(i - SDEPTH))
                fronts[i] = emit_front(b, c, xr, xi)
                if i >= 1:
                    backs[i - 1] = emit_back(fronts.pop(i - 1))
            n = len(pairs)
            backs[n - 1] = emit_back(fronts.pop(n - 1))
            for j in range(n - SDEPTH, n):
                emit_stats(backs.pop(j))

    nc.finalize()
    return nc


_NC_CACHE = None


def _get_nc():
    global _NC_CACHE
    if _NC_CACHE is None:
        _NC_CACHE = build()
    return _NC_CACHE


def make_in_maps(inputs):
    xr = np.ascontiguousarray(
        np.asarray(inputs["x_real"], dtype=np.float32).astype(ml_dtypes.bfloat16))
    xi = np.ascontiguousarray(
        np.asarray(inputs["x_imag"], dtype=np.float32).astype(ml_dtypes.bfloat16))
    wr256 = np.asarray(inputs["Wr256"], dtype=np.float32)
    wi256 = np.asarray(inputs["Wi256"], dtype=np.float32)
    # twiddle-folded stage-1 matrices: M_k1[n2, k2] = W512^(n2*k1) * W256^(n2*k2)
    n256 = np.arange(256)
    w256c = (wr256 + 1j * wi256).T.astype(np.complex64)  # [n2, k2] = W256^(n2 k2)
    tw = np.exp(-2j * np.pi * n256 / 512).astype(np.complex64)
    ms = {}
    for k1 in range(2):
        M = w256c if k1 == 0 else (tw[:, None] * w256c)
        ms[f"M{k1}r"] = np.ascontiguousarray(M.real.astype(ml_dtypes.bfloat16))
        ms[f"M{k1}i"] = np.ascontiguousarray(M.imag.astype(ml_dtypes.bfloat16))
        ms[f"M{k1}ni"] = np.ascontiguousarray((-M.imag).astype(ml_dtypes.bfloat16))
    # stage-2 twiddle-folded: M2_k1p[n2, k2'] = W256^(n2 k1p) * W128^(n2 k2')
    n128 = np.arange(128)
    for k1p in range(2):
        M2 = np.exp(-2j * np.pi * (np.outer(n128, n128) / 128
                                   + k1p * n128[:, None] / 256)
                    ).astype(np.complex64)
        ms[f"W2A{k1p}"] = np.ascontiguousarray(np.concatenate(
            [M2.real, M2.imag], axis=1).astype(ml_dtypes.bfloat16))
        ms[f"W2B{k1p}"] = np.ascontiguousarray(np.concatenate(
            [-M2.imag, M2.real], axis=1).astype(ml_dtypes.bfloat16))
    in_maps = []
    for i in range(NCORES):
        m = {
            "x_real": np.ascontiguousarray(xr[i * BS:(i + 1) * BS]),
            "x_imag": np.ascontiguousarray(xi[i * BS:(i + 1) * BS]),
        }
        m.update(ms)
        in_maps.append(m)
    return in_maps


def run(inputs, trace=False):
    nc = _get_nc()
    in_maps = make_in_maps(inputs)
    try:
        res = run_bass_kernel_spmd(nc, in_maps, list(range(NCORES)),
                                   trace=trace)
    except Exception:
        # transient device wedge (NRT_EXEC_UNIT_UNRECOVERABLE): retry once
        res = run_bass_kernel_spmd(nc, in_maps, list(range(NCORES)),
                                   trace=trace)
    out = np.concatenate([res.results[i]["out"] for i in range(NCORES)],
                         axis=0)
    return out, res


def kernel(**inputs):
    out, _ = run(inputs, trace=False)
    return out


if __name__ == "__main__":
    rng = np.random.default_rng(0)
    ins = {
        "x_real": rng.standard_normal((B, C, R, D)).astype(np.float32),
        "x_imag": rng.standard_normal((B, C, R, D)).astype(np.float32),
    }
    n = np.arange(512)
    W = np.exp(-2j * np.pi * np.outer(n, n) / 512).astype(np.complex64)
    ins["Wr512"], ins["Wi512"] = W.real.copy(), W.imag.copy()
    n = np.arange(256)
    W = np.exp(-2j * np.pi * np.outer(n, n) / 256).astype(np.complex64)
    ins["Wr256"], ins["Wi256"] = W.real.copy(), W.imag.copy()
    out = kernel(**ins)
    print("out", out.shape, out.dtype, float(np.abs(out).mean()))

